# revision 1
# baseline (speedup 1.0000x reference)
"""Trainium2 Bass kernel for nn_CustomFullyConnectedLayerGoogleTopK.

Reference computation:
    a = clip(K * softmax(alpha), 0, 1)                    # (4096,)
    W[rows, cols] += (V * a[:, None])  with rows=(j+i)%N, cols=j
    out = x @ W.T                                          # (256, 4096)

The scatter indices form a bijection (for each col j, row (j+i)%N hits every
row exactly once as i varies), so there is no actual accumulation:

    W[r, c] = V[(r - c) % N, c] * a[(r - c) % N]
    out[b, r] = sum_c x[b, c] * V[(r-c)%N, c] * a[(r-c)%N]

Sharding: output columns r are sharded 8 ways (512 per core) -> no collective;
each core reads only the diagonal band of V it needs (8 MB), all of x (4 MB),
and produces a disjoint out[:, r0:r0+512] slice.

Host-side prep is layout-only (static gather of V's wrapped diagonal band,
x transpose, row reversal, alpha roll + doubling); softmax, clipping, scaling
and the GEMM all run on device. The per-core r0 offset is absorbed into the
input layout (alpha is rolled by r0) so all 8 cores run one SPMD program.

Device-side layout trick: with the contraction rows presented in REVERSED
order (c = N-1-p for SBUF partition-row p), the skewed scale field the band
tiles need becomes the ascending Toeplitz  scale[p, j] = a2[1 + p + j]  where
a2 is `a` doubled. The (doubled, rolled) RAW alpha input is loaded directly in
this overlapping-window layout (one DMA per 4-block batch, partition step +1),
and the softmax scale is applied in place:

    piece = min(exp(alpha_piece + (ln K - ln sum_exp)), 1)
          = min(K * softmax(alpha), 1)     elementwise on the piece

using the otherwise-idle Scalar engine for the biased Exp and GpSimd for the
clip, so nothing round-trips through DRAM and the scale tiles are ready a few
microseconds into the kernel.

The GEMM runs in float32r (full-rate PE mode, ~1.5e-4 rms rel error measured
on HW vs fp64), accumulating fp32 in PSUM over all 32 contraction blocks. The
xT input is declared float32r directly (same 4-byte layout; the PE rounds
internally) so it loads as one plain 4 MB HWDGE DMA with no cast pass.
"""

import math
import os
import sys

import numpy as np

for _p in ("/opt/trn_rl_repo", "/root/.axon_site/_ro/trn_rl_repo"):
    if os.path.isdir(_p) and _p not in sys.path:
        sys.path.append(_p)

import concourse.bacc as bacc
import concourse.bass as bass
import concourse.mybir as mybir
import concourse.tile as tile
from concourse.bass_utils import run_bass_kernel_spmd

F32 = mybir.dt.float32
F32R = mybir.dt.float32r

N = 4096          # IN_F == OUT_F == N_PERM == DIAG
B = 256           # batch
NCORES = 8
RW = N // NCORES  # 512 output columns per core
K_TOPK = 3687     # ceil(0.9 * 4096 * 4096 / 4096)
CB = 128          # contraction block (SBUF partition count)
NCB = N // CB     # 32 contraction blocks
TB = 4            # contraction blocks per DMA/multiply batch
NBATCH = NCB // TB
GPW = RW + (TB - 1) * CB  # 896: width of one Toeplitz scale piece


def _strided_cols(ap2d, col_off, t_step, n_t, inner):
    """[128, W] SBUF tile -> [128, n_t, inner] view starting at col_off with
    column stride t_step between t-slices (overlap allowed)."""
    pstep = ap2d.ap[0][0]
    return bass.AP(
        ap2d.tensor, ap2d.offset + col_off,
        [[pstep, 128], [t_step, n_t], [1, inner]],
    )


def _build_program():
    nc = bacc.Bacc("TRN2", target_bir_lowering=False, debug=False)

    band = nc.dram_tensor("band", [N, RW], F32, kind="ExternalInput").ap()
    xT = nc.dram_tensor("xT", [N, B], F32R, kind="ExternalInput").ap()
    alpha2 = nc.dram_tensor("alpha2", [2 * N], F32, kind="ExternalInput").ap()
    out = nc.dram_tensor("out", [B, RW], F32, kind="ExternalOutput").ap()

    with tile.TileContext(nc) as tc:
        with (
            tc.tile_pool(name="small", bufs=1) as sp,
            tc.tile_pool(name="gpool", bufs=1) as gp,
            tc.tile_pool(name="vb", bufs=6) as vbp,
            tc.tile_pool(name="wt", bufs=4) as wtp,
            tc.tile_pool(name="xtp", bufs=1) as xtp,
            tc.tile_pool(name="opool", bufs=2) as op,
            tc.tile_pool(name="psum", bufs=1, space="PSUM") as pp,
            tc.tile_pool(name="psum_s", bufs=1, space="PSUM") as pps,
        ):
            # ---- softmax normalizer: bias = ln K - ln sum(exp(alpha)) ----
            alpha_sb = sp.tile([128, N // 128], F32)
            nc.gpsimd.dma_start(
                alpha_sb[:], alpha2[0:N].rearrange("(p f) -> p f", p=128)
            )
            # alpha pieces in overlapping Toeplitz layout (no deps: start now)
            ag = []
            for q in range(NBATCH):
                agq = gp.tile([128, GPW], F32, tag=f"g{q}")
                src = bass.AP(
                    alpha2.tensor, alpha2.offset + 1 + q * TB * CB,
                    [[1, 128], [1, GPW]],
                )
                nc.gpsimd.dma_start(agq[:], src)
                ag.append(agq)

            exp_sb = sp.tile([128, N // 128], F32)
            rowsum = sp.tile([128, 1], F32)
            # alpha is uniform in [0,1): no max-subtraction needed for stability
            nc.scalar.activation(
                exp_sb[:], alpha_sb[:], mybir.ActivationFunctionType.Exp,
                accum_out=rowsum[:],
            )
            ones = sp.tile([128, 128], F32)
            nc.vector.memset(ones[:], 1.0)
            tot_ps = pps.tile([128, 1], F32)
            # total = ones.T @ rowsum -> per-partition copy of the full sum
            nc.tensor.matmul(tot_ps[:], ones[:], rowsum[:], start=True, stop=True)
            ln_sum = sp.tile([128, 1], F32)
            nc.scalar.activation(
                ln_sum[:], tot_ps[:], mybir.ActivationFunctionType.Ln
            )
            # bias_neg = ln(sum) - ln(K);  min(K*softmax, 1) = exp(min(z, 0))
            # with z = alpha - bias_neg, and min(z,0) = -relu(-z), so two ACT
            # passes per piece: relu(-alpha + bias_neg) then exp(-that).
            bias_neg = sp.tile([128, 1], F32)
            nc.vector.tensor_scalar_add(
                bias_neg[:], ln_sum[:], -float(math.log(K_TOPK))
            )
            relu_pool = []
            for i in range(2):
                rt_i = sp.tile([128, GPW], F32, tag=f"relu{i}", name=f"relu{i}")
                relu_pool.append(rt_i)
            for q in range(NBATCH):
                rt = relu_pool[q % 2]
                nc.scalar.activation(
                    rt[:], ag[q][:], mybir.ActivationFunctionType.Relu,
                    bias=bias_neg[:, 0:1], scale=-1.0,
                )
                nc.scalar.activation(
                    ag[q][:], rt[:], mybir.ActivationFunctionType.Exp,
                    scale=-1.0,
                )

            # ---- whole xT in one DMA (f32r, plain HWDGE) ----
            xt = xtp.tile([128, NCB, B], F32R)
            nc.scalar.dma_start(
                xt[:], xT.rearrange("(n p) b -> p n b", p=128)
            )

            # ---- main loop: batches of (band * scale) -> matmul pairs ----
            psum0 = pp.tile([128, RW], F32)
            psum1 = pp.tile([128, RW], F32)
            for qi in range(NBATCH):
                q0 = qi * TB
                rows = slice(q0 * CB, (q0 + TB) * CB)
                vb = vbp.tile([128, TB, RW], F32)
                eng = nc.sync if qi < 6 else nc.scalar
                eng.dma_start(
                    vb[:], band[rows, :].rearrange("(t p) j -> p t j", p=128)
                )
                wt = wtp.tile([128, TB, RW], F32R)
                nc.vector.tensor_tensor(
                    wt[:], vb[:], _strided_cols(ag[qi], 0, CB, TB, RW),
                    mybir.AluOpType.mult,
                )
                for t in range(TB):
                    k = q0 + t
                    nc.tensor.matmul(psum0[:], xt[:, k, 0:128], wt[:, t, :],
                                     start=(k == 0), stop=(k == NCB - 1))
                    nc.tensor.matmul(psum1[:], xt[:, k, 128:256], wt[:, t, :],
                                     start=(k == 0), stop=(k == NCB - 1))

            # ---- PSUM -> SBUF -> DRAM ----
            o0 = op.tile([128, RW], F32)
            nc.vector.tensor_copy(o0[:], psum0[:])
            nc.scalar.dma_start(out[0:128, :], o0[:])
            o1 = op.tile([128, RW], F32)
            nc.vector.tensor_copy(o1[:], psum1[:])
            nc.scalar.dma_start(out[128:256, :], o1[:])

    nc.compile()
    return nc


_NC_CACHE = []


def _get_program():
    if not _NC_CACHE:
        _NC_CACHE.append(_build_program())
    return _NC_CACHE[0]


def prepare_in_maps(x: np.ndarray, V: np.ndarray, alpha: np.ndarray):
    """Layout-only sharding of the full inputs into 8 per-core input maps."""
    x = np.ascontiguousarray(np.asarray(x, dtype=np.float32))
    V = np.ascontiguousarray(np.asarray(V, dtype=np.float32))
    alpha = np.ascontiguousarray(np.asarray(alpha, dtype=np.float32))

    # rows presented in reversed order (c = N-1-p); see module docstring
    xT = np.ascontiguousarray(x.T[::-1, :])  # (N, B)

    # VtD[c, t] = V[t % N, c] for t in [0, 2N): doubled transpose for wrap-free
    # band extraction. band_m[c, j] = V[(r0 + j - c) % N, c]
    #              = VtD[c, N + r0 + j - c]
    Vt = np.ascontiguousarray(V.T)
    VtD = np.concatenate([Vt, Vt], axis=1)  # (N, 2N)
    flat = VtD.reshape(-1)
    isz = flat.itemsize

    in_maps = []
    for m in range(NCORES):
        r0 = m * RW
        start = N + r0  # element offset of band_m[0, 0] in flat
        band_m = np.lib.stride_tricks.as_strided(
            flat[start:], shape=(N, RW), strides=((2 * N - 1) * isz, isz),
        )
        am = np.roll(alpha, -r0)
        in_maps.append({
            "band": np.ascontiguousarray(band_m[::-1, :]),
            "xT": xT,
            "alpha2": np.ascontiguousarray(np.concatenate([am, am])),
        })
    return in_maps


def gather_output(results) -> np.ndarray:
    return np.concatenate([results[m]["out"] for m in range(NCORES)], axis=1)


def kernel(x: np.ndarray, V: np.ndarray, alpha: np.ndarray) -> np.ndarray:
    in_maps = prepare_in_maps(x, V, alpha)
    nc = _get_program()
    res = run_bass_kernel_spmd(nc, in_maps, core_ids=list(range(NCORES)))
    return gather_output(res.results)

